# revision 19
# baseline (speedup 1.0000x reference)
"""Trainium2 Bass kernel for nn_BayesianNN (attention + bayesian NEAT scan).

Strategy (8 NeuronCores, SPMD, feature/row tensor-parallel):
  - Shard Wq/Wk/Wv rows (output features) across cores. Per core, W tiles
    stream in fp32, are cast to bf16 (split between ACT and DVE), PE
    transposes them into PSUM banks (8 tiles/bank), one batched copy per
    bank returns them to SBUF, and PE accumulates Q^T/K^T/V^T shards
    = W_shard @ X^T in fp32 PSUM.
  - Partial S = Q^T(shard)^T @ K^T(shard) and partial P = V @ slab
    (slab = mu+sigma*eps rows of the input->hidden block) are both
    computed in phase 1 and AllReduce'd together in one [256,516] fp32
    collective. After softmax, base = (a_bar @ P) needs just two small
    matmuls - no second collective round-trip before the bias add.
  - 260-step topological scan: one tanh per node on ACT
    (v[j] = tanh(v[j-1]*A[j-1,j] + pre[j])), with each node's
    contributions to columns >= j+2 applied off-path by a DVE rank-1
    update into the fp32 pre-activation row. v stays fp32.
"""
import sys

for _p in ("/opt/trn_rl_repo",):
    if _p not in sys.path:
        sys.path.insert(0, _p)

import numpy as np

M = 256
D = 7686
DP = 7808          # D padded to 61*128 for clean 128-chunking
NCH = DP // 128    # 61 d-chunks
HO = 260
NTOT = D + HO
NCORES = 8
SH = 1024          # padded shard rows per core
SCALE = float(1.0 / np.sqrt(np.float32(D)))

SIZES = [961] * 7 + [959]
STARTS = [sum(SIZES[:c]) for c in range(NCORES)]

# d-axis macro chunks (8 chunks of 128 each except the last with 5)
MACROS = [(i * 8, 8) for i in range(7)] + [(56, 5)]  # (chunk0, nchunks)

_CACHE = {}


def _build():
    import concourse.mybir as mybir
    import concourse.tile as tile
    from concourse import bacc
    from concourse.masks import make_identity
    from contextlib import ExitStack

    dt = mybir.dt
    f32, bf = dt.float32, dt.bfloat16
    AF = mybir.ActivationFunctionType
    ALU = mybir.AluOpType
    AX = mybir.AxisListType

    nc = bacc.Bacc(None, target_bir_lowering=False, num_devices=NCORES)

    X = nc.dram_tensor("x", [M, DP], f32, kind="ExternalInput")
    Wq = nc.dram_tensor("wq", [SH, DP], f32, kind="ExternalInput")
    Wk = nc.dram_tensor("wk", [SH, DP], f32, kind="ExternalInput")
    Wv = nc.dram_tensor("wv", [SH, DP], f32, kind="ExternalInput")
    BQ = nc.dram_tensor("bq", [SH], f32, kind="ExternalInput")
    BK = nc.dram_tensor("bk", [SH], f32, kind="ExternalInput")
    BV = nc.dram_tensor("bv", [SH], f32, kind="ExternalInput")
    MUS = nc.dram_tensor("mus", [SH, HO], f32, kind="ExternalInput")
    SGS = nc.dram_tensor("sgs", [SH, HO], f32, kind="ExternalInput")
    EPS = nc.dram_tensor("eps", [SH, HO], f32, kind="ExternalInput")
    MUA = nc.dram_tensor("mua", [HO, HO], f32, kind="ExternalInput")
    SGA = nc.dram_tensor("sga", [HO, HO], f32, kind="ExternalInput")
    EPA = nc.dram_tensor("epa", [HO, HO], f32, kind="ExternalInput")
    BMU = nc.dram_tensor("bmu", [HO], f32, kind="ExternalInput")
    BSG = nc.dram_tensor("bsg", [HO], f32, kind="ExternalInput")
    EPB = nc.dram_tensor("epb", [HO], f32, kind="ExternalInput")
    Y = nc.dram_tensor("y", [4], f32, kind="ExternalOutput")

    RG = [list(range(NCORES))]

    with tile.TileContext(nc) as tc, ExitStack() as ctx:
        const = ctx.enter_context(tc.tile_pool(name="const", bufs=1))
        sm = ctx.enter_context(tc.tile_pool(name="sm", bufs=1))
        vtp = ctx.enter_context(tc.tile_pool(name="vtp", bufs=1))
        scanp = ctx.enter_context(tc.tile_pool(name="scanp", bufs=1))
        dram = ctx.enter_context(tc.tile_pool(name="dram", bufs=1, space="DRAM"))

        idb = const.tile([128, 128], bf, tag="idb")
        make_identity(nc, idb)
        ones_f = const.tile([128, 1], f32, tag="ones_f")
        nc.vector.memset(ones_f[:], 1.0)

        vt_sb = vtp.tile([128, 8, 256], f32, tag="vt_sb")
        slab_sb = vtp.tile([128, 8, HO], f32, tag="slab_sb")

        band = scanp.tile([1, 259, 1], f32, tag="band")
        vrow = scanp.tile([1, HO], f32, tag="vrow")
        pre_sb = scanp.tile([1, HO], f32, tag="pre_sb")
        bb_s = scanp.tile([1, HO], f32, tag="bb_s")
        y4 = scanp.tile([1, 4], f32, tag="y4")

        # ---------- early prep: A block combine + scan tables + slab ----------
        # All prefetch DMAs go through the ACT hwdge queue so the SP queue
        # stays free for the X / W weight stream (keeps PE fed from t=0).
        with tc.tile_pool(name="aprep", bufs=1) as aprep, \
             tc.tile_pool(name="slabl", bufs=2) as slabl:
            aA = aprep.tile([128, 3, HO], f32, tag="aA")
            sA = aprep.tile([128, 3, HO], f32, tag="sA")
            eA = aprep.tile([128, 3, HO], f32, tag="eA")
            nc.scalar.dma_start(aA[:, 0:2, :], MUA[0:256, :].rearrange("(c p) f -> p c f", p=128))
            nc.scalar.dma_start(aA[0:4, 2, :], MUA[256:260, :])
            nc.scalar.dma_start(sA[:, 0:2, :], SGA[0:256, :].rearrange("(c p) f -> p c f", p=128))
            nc.scalar.dma_start(sA[0:4, 2, :], SGA[256:260, :])
            nc.scalar.dma_start(eA[:, 0:2, :], EPA[0:256, :].rearrange("(c p) f -> p c f", p=128))
            nc.scalar.dma_start(eA[0:4, 2, :], EPA[256:260, :])
            nc.vector.tensor_mul(sA[:, 0:2, :], sA[:, 0:2, :], eA[:, 0:2, :])
            nc.vector.tensor_add(aA[:, 0:2, :], aA[:, 0:2, :], sA[:, 0:2, :])
            nc.vector.tensor_mul(sA[0:4, 2, :], sA[0:4, 2, :], eA[0:4, 2, :])
            nc.vector.tensor_add(aA[0:4, 2, :], aA[0:4, 2, :], sA[0:4, 2, :])
            ab = aprep.tile([128, 3, HO], bf, tag="ab")
            nc.vector.tensor_copy(out=ab[:, 0:2, :], in_=aA[:, 0:2, :])
            nc.vector.tensor_copy(out=ab[0:4, 2, :], in_=aA[0:4, 2, :])
            a_dram = dram.tile([HO, HO], bf, tag="a_dram")
            nc.scalar.dma_start(a_dram[0:256, :].rearrange("(c p) f -> p c f", p=128), ab[:, 0:2, :])
            nc.scalar.dma_start(a_dram[256:260, :], ab[0:4, 2, :])
            af_dram = dram.tile([HO, HO], f32, tag="af_dram")
            nc.scalar.dma_start(af_dram[0:256, :].rearrange("(c p) f -> p c f", p=128), aA[:, 0:2, :])
            nc.scalar.dma_start(af_dram[256:260, :], aA[0:4, 2, :])
            # superdiagonal: band[0, k, 0] = A[k, k+1]
            af_flat = af_dram[:].rearrange("a b -> (a b)")
            nc.scalar.dma_start(
                band[:], af_flat[1:1 + 259 * 261].rearrange("(k s) -> k s", s=261)[None, :, 0:1])

            bb_m = aprep.tile([1, HO], f32, tag="bb_m")
            bb_e = aprep.tile([1, HO], f32, tag="bb_e")
            nc.scalar.dma_start(bb_m[:], BMU[:][None, :])
            nc.scalar.dma_start(bb_s[:], BSG[:][None, :])
            nc.scalar.dma_start(bb_e[:], EPB[:][None, :])
            nc.vector.tensor_mul(bb_s[:], bb_s[:], bb_e[:])
            nc.vector.tensor_add(bb_s[:], bb_s[:], bb_m[:])

            for ic in range(8):
                m_t = slabl.tile([128, HO], f32, tag="smu")
                s_t = slabl.tile([128, HO], f32, tag="ssg")
                e_t = slabl.tile([128, HO], f32, tag="sep")
                nc.scalar.dma_start(m_t[:], MUS[ic * 128:(ic + 1) * 128, :])
                nc.scalar.dma_start(s_t[:], SGS[ic * 128:(ic + 1) * 128, :])
                nc.scalar.dma_start(e_t[:], EPS[ic * 128:(ic + 1) * 128, :])
                nc.vector.tensor_mul(s_t[:], s_t[:], e_t[:])
                nc.vector.tensor_add(slab_sb[:, ic, :], m_t[:], s_t[:])

        # ---------- phase 0+1: X^T build, then QKV shard matmuls ----------
        with tc.tile_pool(name="pa_big", bufs=1) as pab, \
             tc.tile_pool(name="wload", bufs=6) as wload, \
             tc.tile_pool(name="wcast", bufs=6) as wcast, \
             tc.tile_pool(name="wtp", bufs=4) as wtp, \
             tc.tile_pool(name="qk", bufs=4) as qk:

            bq_sb = sm.tile([128, 8], f32, tag="bq_sb")
            nc.scalar.dma_start(bq_sb[:], BQ[:].rearrange("(c p) -> p c", p=128))
            bk_sb = sm.tile([128, 8], f32, tag="bk_sb")
            nc.scalar.dma_start(bk_sb[:], BK[:].rearrange("(c p) -> p c", p=128))
            bv_sb = sm.tile([128, 8], f32, tag="bv_sb")
            nc.scalar.dma_start(bv_sb[:], BV[:].rearrange("(c p) -> p c", p=128))

            # xt[d%128, d//128, h*128+m] = X[h*128+m, d] in bf16
            xt = pab.tile([128, NCH, 256], bf, tag="xt")

            with tc.tile_pool(name="ptr", bufs=2, space="PSUM") as ptrp, \
                 tc.tile_pool(name="pacc", bufs=2, space="PSUM") as paccp, \
                 tc.tile_pool(name="ps", bufs=1, space="PSUM") as psp, \
                 tc.tile_pool(name="pp", bufs=1, space="PSUM") as ppp, \
                 tc.tile_pool(name="xbp", bufs=1) as xbp:

                # --- X: per-macro load -> cast -> transpose -> copy ---
                xb = xbp.tile([128, 2, DP], bf, tag="xb")
                for h in range(2):
                    for im, (c0, nch) in enumerate(MACROS):
                        xl = wload.tile([128, 1024], f32, tag="wl")
                        nc.sync.dma_start(xl[:, :nch * 128],
                                          X[h * 128:(h + 1) * 128, c0 * 128:(c0 + nch) * 128])
                        if im % 2 == 0:
                            nc.scalar.copy(out=xb[:, h, c0 * 128:(c0 + nch) * 128],
                                           in_=xl[:, :nch * 128])
                        else:
                            nc.vector.tensor_copy(out=xb[:, h, c0 * 128:(c0 + nch) * 128],
                                                  in_=xl[:, :nch * 128])
                        ptr = ptrp.tile([128, 8, 128], bf, tag="ptr")
                        for c in range(nch):
                            nc.tensor.transpose(ptr[:, c, :], xb[:, h, (c0 + c) * 128:(c0 + c + 1) * 128], idb[:])
                        nc.vector.tensor_copy(out=xt[:, c0:c0 + nch, h * 128:(h + 1) * 128],
                                              in_=ptr[:, 0:nch, :])

                # --- QKV streaming: flat software pipeline over
                # (ic, mat, macro) units; unit u's matmuls are emitted after
                # unit u+1's transposes so the in-order PE never waits on the
                # DVE copy-back of wt, including across mat/ic boundaries.
                s_ps = psp.tile([128, 2, 256], f32, tag="s_ps")
                # P partial: 512-wide halves so each [*, h, 0:260] slice is
                # bank-aligned (1040B used of each 2KB bank)
                p_ps = ppp.tile([128, 2, 512], f32, tag="p_ps")

                MATS = (("q", Wq, bq_sb), ("k", Wk, bk_sb), ("v", Wv, bv_sb))
                qt_tiles = {}
                pending = None  # (acc, c0, nch, wt, ic, mat, bias_sb, is_last)

                def flush(p):
                    acc, c0, nch, wt, ic, mat, bias_sb, is_last = p
                    for c in range(nch):
                        nc.tensor.matmul(acc[:], lhsT=wt[:, c, :], rhs=xt[:, c0 + c, :],
                                         start=(c0 + c == 0), stop=(c0 + c == NCH - 1))
                    if not is_last:
                        return
                    if mat == "v":
                        nc.scalar.activation(out=vt_sb[:, ic, :], in_=acc[:],
                                             func=AF.Identity,
                                             bias=bias_sb[:, ic:ic + 1], scale=1.0)
                        qtq = qt_tiles.pop((ic, "q"))
                        qtk = qt_tiles.pop((ic, "k"))
                        for h in range(2):
                            nc.tensor.matmul(s_ps[:, h, :], lhsT=qtq[:, h * 128:(h + 1) * 128],
                                             rhs=qtk[:], start=(ic == 0 and h == 0),
                                             stop=(ic == 7 and h == 1))
                            nc.tensor.matmul(p_ps[:, h, 0:HO], lhsT=vt_sb[:, ic, h * 128:(h + 1) * 128],
                                             rhs=slab_sb[:, ic, :], start=(ic == 0),
                                             stop=(ic == 7), skip_group_check=True)
                    else:
                        qt = qk.tile([128, 256], bf, tag="qt")
                        nc.scalar.activation(out=qt[:], in_=acc[:], func=AF.Identity,
                                             bias=bias_sb[:, ic:ic + 1], scale=1.0)
                        qt_tiles[(ic, mat)] = qt

                for ic in range(8):
                    for mat, wsrc, bias_sb in MATS:
                        acc = paccp.tile([128, 256], f32, tag="pacc")
                        for im, (c0, nch) in enumerate(MACROS):
                            wl = wload.tile([128, 1024], f32, tag="wl")
                            nc.sync.dma_start(wl[:, :nch * 128],
                                              wsrc[ic * 128:(ic + 1) * 128, c0 * 128:(c0 + nch) * 128])
                            wc = wcast.tile([128, 1024], bf, tag="wc")
                            if im % 3 == 0:
                                nc.vector.tensor_copy(out=wc[:, :nch * 128], in_=wl[:, :nch * 128])
                            else:
                                nc.scalar.copy(out=wc[:, :nch * 128], in_=wl[:, :nch * 128])
                            ptr = ptrp.tile([128, 8, 128], bf, tag="ptr")
                            for c in range(nch):
                                nc.tensor.transpose(ptr[:, c, :], wc[:, c * 128:(c + 1) * 128], idb[:])
                            wt = wtp.tile([128, 8, 128], bf, tag="wt")
                            nc.vector.tensor_copy(out=wt[:, 0:nch, :], in_=ptr[:, 0:nch, :])
                            if pending is not None:
                                flush(pending)
                            pending = (acc, c0, nch, wt, ic, mat, bias_sb,
                                       im == len(MACROS) - 1)
                flush(pending)

                # ---------- phase 2a: fused AllReduce of [S | P] ----------
                sp_in = dram.tile([M, 516], f32, tag="sp_in")
                sp_out = dram.tile([M, 516], f32, tag="sp_out", addr_space="Shared")
                sp_sb = sm.tile([128, 2, 516], f32, tag="sp_sb")
                nc.scalar.copy(out=sp_sb[:, :, 0:256], in_=s_ps[:])
                nc.scalar.copy(out=sp_sb[:, :, 256:516], in_=p_ps[:, :, 0:HO])
                nc.sync.dma_start(sp_in[:].rearrange("(h p) f -> p h f", p=128), sp_sb[:])
                nc.gpsimd.collective_compute("AllReduce", ALU.add, replica_groups=RG,
                                             ins=[sp_in[:].opt()], outs=[sp_out[:].opt()])

        # big phase-1 pools closed: load scan A table now
        abig = ctx.enter_context(tc.tile_pool(name="abig", bufs=1))
        a_p0 = abig.tile([1, HO * HO], bf, tag="a_p0")
        nc.sync.dma_start(a_p0[:], a_dram[:].rearrange("a b -> (a b)")[None, :])

        spr = sm.tile([128, 2, 516], f32, tag="spr")
        nc.sync.dma_start(spr[:], sp_out[:].rearrange("(h p) f -> p h f", p=128))

        # ---------- phase 2b: softmax rows + a_bar + base ----------
        ex = sm.tile([128, 2, 256], f32, tag="ex")
        mx = sm.tile([128, 2], f32, tag="mx")
        nm = sm.tile([128, 2], f32, tag="nm")
        rs = sm.tile([128, 2], f32, tag="rs")
        inv = sm.tile([128, 2], f32, tag="inv")
        for h in range(2):
            nc.vector.tensor_reduce(mx[:, h:h + 1], spr[:, h, 0:256], axis=AX.X, op=ALU.max)
            nc.vector.tensor_scalar_mul(nm[:, h:h + 1], mx[:, h:h + 1], -SCALE)
            nc.scalar.activation(out=ex[:, h, :], in_=spr[:, h, 0:256], func=AF.Exp,
                                 bias=nm[:, h:h + 1], scale=SCALE,
                                 accum_out=rs[:, h:h + 1])
            nc.vector.reciprocal(inv[:, h:h + 1], rs[:, h:h + 1])
            nc.vector.tensor_scalar_mul(ex[:, h, :], ex[:, h, :], inv[:, h:h + 1])

        with tc.tile_pool(name="psm", bufs=2, space="PSUM") as psmp:
            # a_bar as columns: abt[p, mc] = sum_m attn[m, mc*128+p] / M
            abt_ps = psmp.tile([128, 2], f32, tag="abt")
            for mc in range(2):
                for h in range(2):
                    nc.tensor.matmul(abt_ps[:, mc:mc + 1], lhsT=ex[:, h, mc * 128:(mc + 1) * 128],
                                     rhs=ones_f[:], start=(h == 0), stop=(h == 1))
            abt_sb = sm.tile([128, 2], f32, tag="abt_sb")
            nc.scalar.mul(out=abt_sb[:], in_=abt_ps[:], mul=1.0 / M)
            # base = a_bar @ P
            base_ps = psmp.tile([1, HO], f32, tag="base_ps")
            for mc in range(2):
                nc.tensor.matmul(base_ps[0:1, :], lhsT=abt_sb[:, mc:mc + 1],
                                 rhs=spr[:, mc, 256:516], start=(mc == 0), stop=(mc == 1))
            nc.vector.tensor_add(pre_sb[:], base_ps[0:1, :], bb_s[:])

        # ---------- phase 3: 260-step scan ----------
        # v[j] = tanh(v[j-1]*A[j-1,j] + pre[j]); DVE applies v[j]'s
        # contributions to pre[j+2:] off the critical path.
        for j in range(HO):
            if j == 0:
                nc.scalar.activation(out=vrow[0:1, 0:1], in_=pre_sb[0:1, 0:1],
                                     func=AF.Tanh)
            else:
                nc.scalar.activation(out=vrow[0:1, j:j + 1], in_=vrow[0:1, j - 1:j],
                                     func=AF.Tanh,
                                     scale=band[0:1, j - 1:j, 0],
                                     bias=pre_sb[0:1, j:j + 1])
            jl = j + 2
            if jl < HO:
                nc.vector.scalar_tensor_tensor(
                    out=pre_sb[0:1, jl:HO],
                    in0=a_p0[0:1, j * HO + jl:(j + 1) * HO],
                    scalar=vrow[0:1, j:j + 1],
                    in1=pre_sb[0:1, jl:HO],
                    op0=ALU.mult, op1=ALU.add)
        nc.scalar.copy(out=y4[:], in_=vrow[0:1, HO - 4:HO])
        nc.sync.dma_start(Y[:], y4[:])

    nc.compile()
    return nc


def _get_nc():
    if "nc" not in _CACHE:
        _CACHE["nc"] = _build()
    return _CACHE["nc"]


def _make_in_maps(inputs):
    X = np.asarray(inputs["input_matrix"], np.float32)
    Wq = np.asarray(inputs["Wq"], np.float32)
    Wk = np.asarray(inputs["Wk"], np.float32)
    Wv = np.asarray(inputs["Wv"], np.float32)
    bq = np.asarray(inputs["bq"], np.float32)
    bk = np.asarray(inputs["bk"], np.float32)
    bv = np.asarray(inputs["bv"], np.float32)
    mu = np.asarray(inputs["weight_mu"], np.float32)
    sg = np.asarray(inputs["weight_sigma"], np.float32)
    ep = np.asarray(inputs["eps_w"], np.float32)
    bmu = np.asarray(inputs["bias_mu"], np.float32)
    bsg = np.asarray(inputs["bias_sigma"], np.float32)
    epb = np.asarray(inputs["eps_b"], np.float32)

    Xp = np.zeros((M, DP), np.float32)
    Xp[:, :D] = X

    muA = np.ascontiguousarray(mu[D:NTOT, D:NTOT])
    sgA = np.ascontiguousarray(sg[D:NTOT, D:NTOT])
    epA = np.ascontiguousarray(ep[D:NTOT, D:NTOT])

    in_maps = []
    for c in range(NCORES):
        st, sz = STARTS[c], SIZES[c]

        def rows2d_pad(A):
            out = np.zeros((SH, DP), np.float32)
            out[:sz, :D] = A[st:st + sz]
            return out

        def rows2d(A):
            out = np.zeros((SH, A.shape[1]), np.float32)
            out[:sz] = A[st:st + sz]
            return out

        def rows1d(a):
            out = np.zeros((SH,), np.float32)
            out[:sz] = a[st:st + sz]
            return out

        in_maps.append({
            "x": Xp,
            "wq": rows2d_pad(Wq), "wk": rows2d_pad(Wk), "wv": rows2d_pad(Wv),
            "bq": rows1d(bq), "bk": rows1d(bk), "bv": rows1d(bv),
            "mus": rows2d(mu[:, D:NTOT]),
            "sgs": rows2d(sg[:, D:NTOT]),
            "eps": rows2d(ep[:, D:NTOT]),
            "mua": muA, "sga": sgA, "epa": epA,
            "bmu": bmu, "bsg": bsg, "epb": epb,
        })
    return in_maps


def kernel(**inputs):
    from concourse.bass_utils import run_bass_kernel_spmd

    nc = _get_nc()
    in_maps = _make_in_maps(inputs)
    res = run_bass_kernel_spmd(nc, in_maps, core_ids=list(range(NCORES)))
    return np.asarray(res.results[0]["y"], np.float32).reshape(4)


# revision 23
# speedup vs baseline: 1.0135x; 1.0135x over previous
"""Trainium2 Bass kernel for nn_BayesianNN (attention + bayesian NEAT scan).

Strategy (8 NeuronCores, SPMD, feature/row tensor-parallel):
  - Shard Wq/Wk/Wv rows (output features) across cores. Per core, W tiles
    stream in fp32, are cast to bf16 (split between ACT and DVE), PE
    transposes them into PSUM banks (8 tiles/bank), one batched copy per
    bank returns them to SBUF, and PE accumulates Q^T/K^T/V^T shards
    = W_shard @ X^T in fp32 PSUM.
  - Partial S = Q^T(shard)^T @ K^T(shard) and partial P = V @ slab
    (slab = mu+sigma*eps rows of the input->hidden block) are both
    computed in phase 1 and AllReduce'd together in one [256,516] fp32
    collective. After softmax, base = (a_bar @ P) needs just two small
    matmuls - no second collective round-trip before the bias add.
  - 260-step topological scan: one tanh per node on ACT
    (v[j] = tanh(v[j-1]*A[j-1,j] + pre[j])), with each node's
    contributions to columns >= j+2 applied off-path by a DVE rank-1
    update into the fp32 pre-activation row. v stays fp32.
"""
import sys

for _p in ("/opt/trn_rl_repo",):
    if _p not in sys.path:
        sys.path.insert(0, _p)

import numpy as np

M = 256
D = 7686
DP = 7808          # D padded to 61*128 for clean 128-chunking
NCH = DP // 128    # 61 d-chunks
HO = 260
NTOT = D + HO
NCORES = 8
SH = 1024          # padded shard rows per core
SCALE = float(1.0 / np.sqrt(np.float32(D)))

SIZES = [961] * 7 + [959]
STARTS = [sum(SIZES[:c]) for c in range(NCORES)]

# d-axis macro chunks (8 chunks of 128 each except the last with 5)
MACROS = [(i * 8, 8) for i in range(7)] + [(56, 5)]  # (chunk0, nchunks)

_CACHE = {}


def _build():
    import concourse.mybir as mybir
    import concourse.tile as tile
    from concourse import bacc
    from concourse.masks import make_identity
    from contextlib import ExitStack

    dt = mybir.dt
    f32, bf = dt.float32, dt.bfloat16
    AF = mybir.ActivationFunctionType
    ALU = mybir.AluOpType
    AX = mybir.AxisListType

    nc = bacc.Bacc(None, target_bir_lowering=False, num_devices=NCORES)

    X = nc.dram_tensor("x", [M, DP], f32, kind="ExternalInput")
    Wq = nc.dram_tensor("wq", [SH, DP], f32, kind="ExternalInput")
    Wk = nc.dram_tensor("wk", [SH, DP], f32, kind="ExternalInput")
    Wv = nc.dram_tensor("wv", [SH, DP], f32, kind="ExternalInput")
    BQ = nc.dram_tensor("bq", [SH], f32, kind="ExternalInput")
    BK = nc.dram_tensor("bk", [SH], f32, kind="ExternalInput")
    BV = nc.dram_tensor("bv", [SH], f32, kind="ExternalInput")
    MUS = nc.dram_tensor("mus", [SH, HO], f32, kind="ExternalInput")
    SGS = nc.dram_tensor("sgs", [SH, HO], f32, kind="ExternalInput")
    EPS = nc.dram_tensor("eps", [SH, HO], f32, kind="ExternalInput")
    MUA = nc.dram_tensor("mua", [HO, HO], f32, kind="ExternalInput")
    SGA = nc.dram_tensor("sga", [HO, HO], f32, kind="ExternalInput")
    EPA = nc.dram_tensor("epa", [HO, HO], f32, kind="ExternalInput")
    BMU = nc.dram_tensor("bmu", [HO], f32, kind="ExternalInput")
    BSG = nc.dram_tensor("bsg", [HO], f32, kind="ExternalInput")
    EPB = nc.dram_tensor("epb", [HO], f32, kind="ExternalInput")
    Y = nc.dram_tensor("y", [4], f32, kind="ExternalOutput")

    RG = [list(range(NCORES))]

    with tile.TileContext(nc) as tc, ExitStack() as ctx:
        const = ctx.enter_context(tc.tile_pool(name="const", bufs=1))
        sm = ctx.enter_context(tc.tile_pool(name="sm", bufs=1))
        vtp = ctx.enter_context(tc.tile_pool(name="vtp", bufs=1))
        scanp = ctx.enter_context(tc.tile_pool(name="scanp", bufs=1))
        dram = ctx.enter_context(tc.tile_pool(name="dram", bufs=1, space="DRAM"))

        idb = const.tile([128, 128], bf, tag="idb")
        make_identity(nc, idb)
        ones_f = const.tile([128, 1], f32, tag="ones_f")
        nc.vector.memset(ones_f[:], 1.0)

        vt_sb = vtp.tile([128, 8, 256], f32, tag="vt_sb")
        slab_sb = vtp.tile([128, 8, HO], f32, tag="slab_sb")

        band = scanp.tile([1, 259, 1], f32, tag="band")
        vrow = scanp.tile([1, HO], f32, tag="vrow")
        pre_sb = scanp.tile([1, HO], f32, tag="pre_sb")
        bb_s = scanp.tile([1, HO], f32, tag="bb_s")
        y4 = scanp.tile([1, 4], f32, tag="y4")

        # ---------- early prep: A block combine + scan tables + slab ----------
        # All prefetch DMAs go through the ACT hwdge queue so the SP queue
        # stays free for the X / W weight stream (keeps PE fed from t=0).
        with tc.tile_pool(name="aprep", bufs=1) as aprep, \
             tc.tile_pool(name="slabl", bufs=2) as slabl:
            aA = aprep.tile([128, 3, HO], f32, tag="aA")
            sA = aprep.tile([128, 3, HO], f32, tag="sA")
            eA = aprep.tile([128, 3, HO], f32, tag="eA")
            nc.scalar.dma_start(aA[:, 0:2, :], MUA[0:256, :].rearrange("(c p) f -> p c f", p=128))
            nc.scalar.dma_start(aA[0:4, 2, :], MUA[256:260, :])
            nc.scalar.dma_start(sA[:, 0:2, :], SGA[0:256, :].rearrange("(c p) f -> p c f", p=128))
            nc.scalar.dma_start(sA[0:4, 2, :], SGA[256:260, :])
            nc.scalar.dma_start(eA[:, 0:2, :], EPA[0:256, :].rearrange("(c p) f -> p c f", p=128))
            nc.scalar.dma_start(eA[0:4, 2, :], EPA[256:260, :])
            nc.vector.tensor_mul(sA[:, 0:2, :], sA[:, 0:2, :], eA[:, 0:2, :])
            nc.vector.tensor_add(aA[:, 0:2, :], aA[:, 0:2, :], sA[:, 0:2, :])
            nc.vector.tensor_mul(sA[0:4, 2, :], sA[0:4, 2, :], eA[0:4, 2, :])
            nc.vector.tensor_add(aA[0:4, 2, :], aA[0:4, 2, :], sA[0:4, 2, :])
            ab = aprep.tile([128, 3, HO], bf, tag="ab")
            nc.vector.tensor_copy(out=ab[:, 0:2, :], in_=aA[:, 0:2, :])
            nc.vector.tensor_copy(out=ab[0:4, 2, :], in_=aA[0:4, 2, :])
            a_dram = dram.tile([HO, HO], bf, tag="a_dram")
            nc.scalar.dma_start(a_dram[0:256, :].rearrange("(c p) f -> p c f", p=128), ab[:, 0:2, :])
            nc.scalar.dma_start(a_dram[256:260, :], ab[0:4, 2, :])
            af_dram = dram.tile([HO, HO], f32, tag="af_dram")
            nc.scalar.dma_start(af_dram[0:256, :].rearrange("(c p) f -> p c f", p=128), aA[:, 0:2, :])
            nc.scalar.dma_start(af_dram[256:260, :], aA[0:4, 2, :])
            # superdiagonal: band[0, k, 0] = A[k, k+1]
            af_flat = af_dram[:].rearrange("a b -> (a b)")
            nc.scalar.dma_start(
                band[:], af_flat[1:1 + 259 * 261].rearrange("(k s) -> k s", s=261)[None, :, 0:1])

            bb_m = aprep.tile([1, HO], f32, tag="bb_m")
            bb_e = aprep.tile([1, HO], f32, tag="bb_e")
            nc.scalar.dma_start(bb_m[:], BMU[:][None, :])
            nc.scalar.dma_start(bb_s[:], BSG[:][None, :])
            nc.scalar.dma_start(bb_e[:], EPB[:][None, :])
            nc.vector.tensor_mul(bb_s[:], bb_s[:], bb_e[:])
            nc.vector.tensor_add(bb_s[:], bb_s[:], bb_m[:])

            for ic in range(8):
                m_t = slabl.tile([128, HO], f32, tag="smu")
                s_t = slabl.tile([128, HO], f32, tag="ssg")
                e_t = slabl.tile([128, HO], f32, tag="sep")
                nc.scalar.dma_start(m_t[:], MUS[ic * 128:(ic + 1) * 128, :])
                nc.scalar.dma_start(s_t[:], SGS[ic * 128:(ic + 1) * 128, :])
                nc.scalar.dma_start(e_t[:], EPS[ic * 128:(ic + 1) * 128, :])
                nc.vector.tensor_mul(s_t[:], s_t[:], e_t[:])
                nc.vector.tensor_add(slab_sb[:, ic, :], m_t[:], s_t[:])

        # ---------- phase 0+1: X^T build, then QKV shard matmuls ----------
        with tc.tile_pool(name="pa_big", bufs=1) as pab, \
             tc.tile_pool(name="wload", bufs=6) as wload, \
             tc.tile_pool(name="wcast", bufs=6) as wcast, \
             tc.tile_pool(name="wtp", bufs=4) as wtp, \
             tc.tile_pool(name="qk", bufs=4) as qk:

            bq_sb = sm.tile([128, 8], f32, tag="bq_sb")
            nc.scalar.dma_start(bq_sb[:], BQ[:].rearrange("(c p) -> p c", p=128))
            bk_sb = sm.tile([128, 8], f32, tag="bk_sb")
            nc.scalar.dma_start(bk_sb[:], BK[:].rearrange("(c p) -> p c", p=128))
            bv_sb = sm.tile([128, 8], f32, tag="bv_sb")
            nc.scalar.dma_start(bv_sb[:], BV[:].rearrange("(c p) -> p c", p=128))

            # xt[d%128, d//128, h*128+m] = X[h*128+m, d] in bf16
            xt = pab.tile([128, NCH, 256], bf, tag="xt")

            with tc.tile_pool(name="ptr", bufs=2, space="PSUM") as ptrp, \
                 tc.tile_pool(name="pacc", bufs=2, space="PSUM") as paccp, \
                 tc.tile_pool(name="ps", bufs=1, space="PSUM") as psp, \
                 tc.tile_pool(name="pp", bufs=1, space="PSUM") as ppp, \
                 tc.tile_pool(name="xbp", bufs=1) as xbp:

                # --- X: per-macro load -> cast -> transpose -> copy ---
                xb = xbp.tile([128, 2, DP], bf, tag="xb")
                for h in range(2):
                    for im, (c0, nch) in enumerate(MACROS):
                        xl = wload.tile([128, 1024], f32, tag="wl")
                        nc.sync.dma_start(xl[:, :nch * 128],
                                          X[h * 128:(h + 1) * 128, c0 * 128:(c0 + nch) * 128])
                        if im % 2 == 0:
                            nc.scalar.copy(out=xb[:, h, c0 * 128:(c0 + nch) * 128],
                                           in_=xl[:, :nch * 128])
                        else:
                            nc.vector.tensor_copy(out=xb[:, h, c0 * 128:(c0 + nch) * 128],
                                                  in_=xl[:, :nch * 128])
                        ptr = ptrp.tile([128, 8, 128], bf, tag="ptr")
                        for c in range(nch):
                            nc.tensor.transpose(ptr[:, c, :], xb[:, h, (c0 + c) * 128:(c0 + c + 1) * 128], idb[:])
                        nc.vector.tensor_copy(out=xt[:, c0:c0 + nch, h * 128:(h + 1) * 128],
                                              in_=ptr[:, 0:nch, :])

                # --- QKV streaming: flat software pipeline over
                # (ic, mat, macro) units; unit u's matmuls are emitted after
                # unit u+1's transposes so the in-order PE never waits on the
                # DVE copy-back of wt, including across mat/ic boundaries.
                s_ps = psp.tile([128, 2, 256], f32, tag="s_ps")
                # P partial: 512-wide halves so each [*, h, 0:260] slice is
                # bank-aligned (1040B used of each 2KB bank)
                p_ps = ppp.tile([128, 2, 512], f32, tag="p_ps")

                MATS = (("q", Wq, bq_sb), ("k", Wk, bk_sb), ("v", Wv, bv_sb))
                qt_tiles = {}
                pending = None  # (acc, c0, nch, wt, ic, mat, bias_sb, is_last)

                def flush(p):
                    acc, c0, nch, wt, ic, mat, bias_sb, is_last = p
                    for c in range(nch):
                        nc.tensor.matmul(acc[:], lhsT=wt[:, c, :], rhs=xt[:, c0 + c, :],
                                         start=(c0 + c == 0), stop=(c0 + c == NCH - 1))
                    if not is_last:
                        return
                    if mat == "v":
                        nc.scalar.activation(out=vt_sb[:, ic, :], in_=acc[:],
                                             func=AF.Identity,
                                             bias=bias_sb[:, ic:ic + 1], scale=1.0)
                        qtq = qt_tiles.pop((ic, "q"))
                        qtk = qt_tiles.pop((ic, "k"))
                        for h in range(2):
                            nc.tensor.matmul(s_ps[:, h, :], lhsT=qtq[:, h * 128:(h + 1) * 128],
                                             rhs=qtk[:], start=(ic == 0 and h == 0),
                                             stop=(ic == 7 and h == 1))
                            nc.tensor.matmul(p_ps[:, h, 0:HO], lhsT=vt_sb[:, ic, h * 128:(h + 1) * 128],
                                             rhs=slab_sb[:, ic, :], start=(ic == 0),
                                             stop=(ic == 7), skip_group_check=True)
                    else:
                        qt = qk.tile([128, 256], bf, tag="qt")
                        nc.scalar.activation(out=qt[:], in_=acc[:], func=AF.Identity,
                                             bias=bias_sb[:, ic:ic + 1], scale=1.0)
                        qt_tiles[(ic, mat)] = qt

                for ic in range(8):
                    for mat, wsrc, bias_sb in MATS:
                        acc = paccp.tile([128, 256], f32, tag="pacc")
                        for im, (c0, nch) in enumerate(MACROS):
                            wl = wload.tile([128, 1024], f32, tag="wl")
                            nc.sync.dma_start(wl[:, :nch * 128],
                                              wsrc[ic * 128:(ic + 1) * 128, c0 * 128:(c0 + nch) * 128])
                            wc = wcast.tile([128, 1024], bf, tag="wc")
                            if im % 3 == 0:
                                nc.vector.tensor_copy(out=wc[:, :nch * 128], in_=wl[:, :nch * 128])
                            else:
                                nc.scalar.copy(out=wc[:, :nch * 128], in_=wl[:, :nch * 128])
                            ptr = ptrp.tile([128, 8, 128], bf, tag="ptr")
                            for c in range(nch):
                                nc.tensor.transpose(ptr[:, c, :], wc[:, c * 128:(c + 1) * 128], idb[:])
                            wt = wtp.tile([128, 8, 128], bf, tag="wt")
                            nc.vector.tensor_copy(out=wt[:, 0:nch, :], in_=ptr[:, 0:nch, :])
                            if pending is not None:
                                flush(pending)
                            pending = (acc, c0, nch, wt, ic, mat, bias_sb,
                                       im == len(MACROS) - 1)
                flush(pending)

                # ---------- phase 2a: fused AllReduce of [S | P] ----------
                sp_in = dram.tile([M, 516], f32, tag="sp_in")
                sp_out = dram.tile([M, 516], f32, tag="sp_out", addr_space="Shared")
                sp_sb = sm.tile([128, 2, 516], f32, tag="sp_sb")
                nc.scalar.copy(out=sp_sb[:, :, 0:256], in_=s_ps[:])
                nc.scalar.copy(out=sp_sb[:, :, 256:516], in_=p_ps[:, :, 0:HO])
                nc.sync.dma_start(sp_in[:].rearrange("(h p) f -> p h f", p=128), sp_sb[:])
                nc.gpsimd.collective_compute("AllReduce", ALU.add, replica_groups=RG,
                                             ins=[sp_in[:].opt()], outs=[sp_out[:].opt()])

        # big phase-1 pools closed: load scan A table now
        abig = ctx.enter_context(tc.tile_pool(name="abig", bufs=1))
        a_p0 = abig.tile([1, HO * HO], bf, tag="a_p0")
        nc.sync.dma_start(a_p0[:], a_dram[:].rearrange("a b -> (a b)")[None, :])

        spr = sm.tile([128, 2, 516], f32, tag="spr")
        nc.sync.dma_start(spr[:], sp_out[:].rearrange("(h p) f -> p h f", p=128))

        # ---------- phase 2b: softmax rows + a_bar + base ----------
        ex = sm.tile([128, 2, 256], f32, tag="ex")
        mx = sm.tile([128, 2], f32, tag="mx")
        nm = sm.tile([128, 2], f32, tag="nm")
        rs = sm.tile([128, 2], f32, tag="rs")
        inv = sm.tile([128, 2], f32, tag="inv")
        for h in range(2):
            nc.vector.tensor_reduce(mx[:, h:h + 1], spr[:, h, 0:256], axis=AX.X, op=ALU.max)
            nc.vector.tensor_scalar_mul(nm[:, h:h + 1], mx[:, h:h + 1], -SCALE)
            nc.scalar.activation(out=ex[:, h, :], in_=spr[:, h, 0:256], func=AF.Exp,
                                 bias=nm[:, h:h + 1], scale=SCALE,
                                 accum_out=rs[:, h:h + 1])
            nc.vector.reciprocal(inv[:, h:h + 1], rs[:, h:h + 1])
            nc.vector.tensor_scalar_mul(ex[:, h, :], ex[:, h, :], inv[:, h:h + 1])

        with tc.tile_pool(name="psm", bufs=2, space="PSUM") as psmp:
            # a_bar as columns: abt[p, mc] = sum_m attn[m, mc*128+p] / M
            abt_ps = psmp.tile([128, 2], f32, tag="abt")
            for mc in range(2):
                for h in range(2):
                    nc.tensor.matmul(abt_ps[:, mc:mc + 1], lhsT=ex[:, h, mc * 128:(mc + 1) * 128],
                                     rhs=ones_f[:], start=(h == 0), stop=(h == 1))
            abt_sb = sm.tile([128, 2], f32, tag="abt_sb")
            nc.scalar.mul(out=abt_sb[:], in_=abt_ps[:], mul=1.0 / M)
            # base = a_bar @ P
            base_ps = psmp.tile([1, HO], f32, tag="base_ps")
            for mc in range(2):
                nc.tensor.matmul(base_ps[0:1, :], lhsT=abt_sb[:, mc:mc + 1],
                                 rhs=spr[:, mc, 256:516], start=(mc == 0), stop=(mc == 1))
            nc.vector.tensor_add(pre_sb[:], base_ps[0:1, :], bb_s[:])

        # ---------- phase 3: 260-step scan ----------
        # v[j] = tanh(v[j-1]*A[j-1,j] + pre[j]); DVE applies v[j]'s
        # contributions to pre[j+2:] off the critical path.
        for j in range(HO):
            if j == 0:
                nc.scalar.activation(out=vrow[0:1, 0:1], in_=pre_sb[0:1, 0:1],
                                     func=AF.Tanh)
            else:
                nc.scalar.activation(out=vrow[0:1, j:j + 1], in_=vrow[0:1, j - 1:j],
                                     func=AF.Tanh,
                                     scale=band[0:1, j - 1:j, 0],
                                     bias=pre_sb[0:1, j:j + 1])
            jl = j + 2
            if jl < HO:
                nc.vector.scalar_tensor_tensor(
                    out=pre_sb[0:1, jl:HO],
                    in0=a_p0[0:1, j * HO + jl:(j + 1) * HO],
                    scalar=vrow[0:1, j:j + 1],
                    in1=pre_sb[0:1, jl:HO],
                    op0=ALU.mult, op1=ALU.add)
        nc.scalar.copy(out=y4[:], in_=vrow[0:1, HO - 4:HO])
        nc.sync.dma_start(Y[:], y4[:])

    nc.compile()
    return nc


def _get_nc():
    if "nc" not in _CACHE:
        _CACHE["nc"] = _build()
    return _CACHE["nc"]


def _make_in_maps(inputs):
    X = np.asarray(inputs["input_matrix"], np.float32)
    Wq = np.asarray(inputs["Wq"], np.float32)
    Wk = np.asarray(inputs["Wk"], np.float32)
    Wv = np.asarray(inputs["Wv"], np.float32)
    bq = np.asarray(inputs["bq"], np.float32)
    bk = np.asarray(inputs["bk"], np.float32)
    bv = np.asarray(inputs["bv"], np.float32)
    mu = np.asarray(inputs["weight_mu"], np.float32)
    sg = np.asarray(inputs["weight_sigma"], np.float32)
    ep = np.asarray(inputs["eps_w"], np.float32)
    bmu = np.asarray(inputs["bias_mu"], np.float32)
    bsg = np.asarray(inputs["bias_sigma"], np.float32)
    epb = np.asarray(inputs["eps_b"], np.float32)

    Xp = np.zeros((M, DP), np.float32)
    Xp[:, :D] = X

    muA = np.ascontiguousarray(mu[D:NTOT, D:NTOT])
    sgA = np.ascontiguousarray(sg[D:NTOT, D:NTOT])
    epA = np.ascontiguousarray(ep[D:NTOT, D:NTOT])

    in_maps = []
    for c in range(NCORES):
        st, sz = STARTS[c], SIZES[c]

        def rows2d_pad(A):
            out = np.zeros((SH, DP), np.float32)
            out[:sz, :D] = A[st:st + sz]
            return out

        def rows2d(A):
            out = np.zeros((SH, A.shape[1]), np.float32)
            out[:sz] = A[st:st + sz]
            return out

        def rows1d(a):
            out = np.zeros((SH,), np.float32)
            out[:sz] = a[st:st + sz]
            return out

        in_maps.append({
            "x": Xp,
            "wq": rows2d_pad(Wq), "wk": rows2d_pad(Wk), "wv": rows2d_pad(Wv),
            "bq": rows1d(bq), "bk": rows1d(bk), "bv": rows1d(bv),
            "mus": rows2d(mu[:, D:NTOT]),
            "sgs": rows2d(sg[:, D:NTOT]),
            "eps": rows2d(ep[:, D:NTOT]),
            "mua": muA, "sga": sgA, "epa": epA,
            "bmu": bmu, "bsg": bsg, "epb": epb,
        })
    return in_maps


def kernel(**inputs):
    from concourse.bass_utils import run_bass_kernel_spmd

    nc = _get_nc()
    in_maps = _make_in_maps(inputs)
    res = run_bass_kernel_spmd(nc, in_maps, core_ids=list(range(NCORES)))
    return np.asarray(res.results[0]["y"], np.float32).reshape(4)


# revision 27
# speedup vs baseline: 1.0452x; 1.0313x over previous
"""Trainium2 Bass kernel for nn_BayesianNN (attention + bayesian NEAT scan).

Strategy (8 NeuronCores, SPMD, feature/row tensor-parallel):
  - Shard Wq/Wk/Wv rows (output features) across cores. Per core, W tiles
    stream in fp32, are cast to bf16 (split between ACT and DVE), PE
    transposes them into PSUM banks (8 tiles/bank), one batched copy per
    bank returns them to SBUF, and PE accumulates Q^T/K^T/V^T shards
    = W_shard @ X^T in fp32 PSUM.
  - Partial S = Q^T(shard)^T @ K^T(shard) and partial P = V @ slab
    (slab = mu+sigma*eps rows of the input->hidden block) are both
    computed in phase 1 and AllReduce'd together in one [256,516] fp32
    collective. After softmax, base = (a_bar @ P) needs just two small
    matmuls - no second collective round-trip before the bias add.
  - 260-step topological scan: one tanh per node on ACT
    (v[j] = tanh(v[j-1]*A[j-1,j] + pre[j])), with each node's
    contributions to columns >= j+2 applied off-path by a DVE rank-1
    update into the fp32 pre-activation row. v stays fp32.
"""
import sys

for _p in ("/opt/trn_rl_repo",):
    if _p not in sys.path:
        sys.path.insert(0, _p)

import numpy as np

M = 256
D = 7686
DP = 7808          # D padded to 61*128 for clean 128-chunking
NCH = DP // 128    # 61 d-chunks
HO = 260
NTOT = D + HO
NCORES = 8
SH = 1024          # padded shard rows per core
SCALE = float(1.0 / np.sqrt(np.float32(D)))

SIZES = [961] * 7 + [959]
STARTS = [sum(SIZES[:c]) for c in range(NCORES)]

# d-axis macro chunks (8 chunks of 128 each except the last with 5)
MACROS = [(i * 8, 8) for i in range(7)] + [(56, 5)]  # (chunk0, nchunks)

_CACHE = {}


def _build():
    import concourse.mybir as mybir
    import concourse.tile as tile
    from concourse import bacc
    from concourse.masks import make_identity
    from contextlib import ExitStack

    dt = mybir.dt
    f32, bf = dt.float32, dt.bfloat16
    AF = mybir.ActivationFunctionType
    ALU = mybir.AluOpType
    AX = mybir.AxisListType

    nc = bacc.Bacc(None, target_bir_lowering=False, num_devices=NCORES)

    X = nc.dram_tensor("x", [M, DP], f32, kind="ExternalInput")
    Wq = nc.dram_tensor("wq", [SH, DP], f32, kind="ExternalInput")
    Wk = nc.dram_tensor("wk", [SH, DP], f32, kind="ExternalInput")
    Wv = nc.dram_tensor("wv", [SH, DP], f32, kind="ExternalInput")
    BQ = nc.dram_tensor("bq", [SH], f32, kind="ExternalInput")
    BK = nc.dram_tensor("bk", [SH], f32, kind="ExternalInput")
    BV = nc.dram_tensor("bv", [SH], f32, kind="ExternalInput")
    MUS = nc.dram_tensor("mus", [SH, HO], f32, kind="ExternalInput")
    SGS = nc.dram_tensor("sgs", [SH, HO], f32, kind="ExternalInput")
    EPS = nc.dram_tensor("eps", [SH, HO], f32, kind="ExternalInput")
    MUA = nc.dram_tensor("mua", [HO, HO], f32, kind="ExternalInput")
    SGA = nc.dram_tensor("sga", [HO, HO], f32, kind="ExternalInput")
    EPA = nc.dram_tensor("epa", [HO, HO], f32, kind="ExternalInput")
    BMU = nc.dram_tensor("bmu", [HO], f32, kind="ExternalInput")
    BSG = nc.dram_tensor("bsg", [HO], f32, kind="ExternalInput")
    EPB = nc.dram_tensor("epb", [HO], f32, kind="ExternalInput")
    Y = nc.dram_tensor("y", [4], f32, kind="ExternalOutput")

    RG = [list(range(NCORES))]

    with tile.TileContext(nc) as tc, ExitStack() as ctx:
        const = ctx.enter_context(tc.tile_pool(name="const", bufs=1))
        sm = ctx.enter_context(tc.tile_pool(name="sm", bufs=1))
        vtp = ctx.enter_context(tc.tile_pool(name="vtp", bufs=1))
        scanp = ctx.enter_context(tc.tile_pool(name="scanp", bufs=1))
        dram = ctx.enter_context(tc.tile_pool(name="dram", bufs=1, space="DRAM"))

        idb = const.tile([128, 128], bf, tag="idb")
        make_identity(nc, idb)
        ones_f = const.tile([128, 1], f32, tag="ones_f")
        nc.vector.memset(ones_f[:], 1.0)

        vt_sb = vtp.tile([128, 8, 256], f32, tag="vt_sb")
        slab_sb = vtp.tile([128, 8, HO], f32, tag="slab_sb")

        band = scanp.tile([1, 259, 1], f32, tag="band")
        vrow = scanp.tile([1, HO], bf, tag="vrow")
        pre_sb = scanp.tile([1, HO], f32, tag="pre_sb")
        bb_s = scanp.tile([1, HO], f32, tag="bb_s")
        y4 = scanp.tile([1, 4], f32, tag="y4")

        # ---------- early prep: A block combine + scan tables + slab ----------
        # All prefetch DMAs go through the ACT hwdge queue so the SP queue
        # stays free for the X / W weight stream (keeps PE fed from t=0).
        with tc.tile_pool(name="aprep", bufs=1) as aprep, \
             tc.tile_pool(name="slabl", bufs=2) as slabl:
            aA = aprep.tile([128, 3, HO], f32, tag="aA")
            sA = aprep.tile([128, 3, HO], f32, tag="sA")
            eA = aprep.tile([128, 3, HO], f32, tag="eA")
            nc.scalar.dma_start(aA[:, 0:2, :], MUA[0:256, :].rearrange("(c p) f -> p c f", p=128))
            nc.scalar.dma_start(aA[0:4, 2, :], MUA[256:260, :])
            nc.scalar.dma_start(sA[:, 0:2, :], SGA[0:256, :].rearrange("(c p) f -> p c f", p=128))
            nc.scalar.dma_start(sA[0:4, 2, :], SGA[256:260, :])
            nc.scalar.dma_start(eA[:, 0:2, :], EPA[0:256, :].rearrange("(c p) f -> p c f", p=128))
            nc.scalar.dma_start(eA[0:4, 2, :], EPA[256:260, :])
            nc.vector.tensor_mul(sA[:, 0:2, :], sA[:, 0:2, :], eA[:, 0:2, :])
            nc.vector.tensor_add(aA[:, 0:2, :], aA[:, 0:2, :], sA[:, 0:2, :])
            nc.vector.tensor_mul(sA[0:4, 2, :], sA[0:4, 2, :], eA[0:4, 2, :])
            nc.vector.tensor_add(aA[0:4, 2, :], aA[0:4, 2, :], sA[0:4, 2, :])
            ab = aprep.tile([128, 3, HO], bf, tag="ab")
            nc.vector.tensor_copy(out=ab[:, 0:2, :], in_=aA[:, 0:2, :])
            nc.vector.tensor_copy(out=ab[0:4, 2, :], in_=aA[0:4, 2, :])
            a_dram = dram.tile([HO, HO], bf, tag="a_dram")
            nc.scalar.dma_start(a_dram[0:256, :].rearrange("(c p) f -> p c f", p=128), ab[:, 0:2, :])
            nc.scalar.dma_start(a_dram[256:260, :], ab[0:4, 2, :])
            af_dram = dram.tile([HO, HO], f32, tag="af_dram")
            nc.scalar.dma_start(af_dram[0:256, :].rearrange("(c p) f -> p c f", p=128), aA[:, 0:2, :])
            nc.scalar.dma_start(af_dram[256:260, :], aA[0:4, 2, :])
            # superdiagonal: band[0, k, 0] = A[k, k+1]
            af_flat = af_dram[:].rearrange("a b -> (a b)")
            nc.scalar.dma_start(
                band[:], af_flat[1:1 + 259 * 261].rearrange("(k s) -> k s", s=261)[None, :, 0:1])

            bb_m = aprep.tile([1, HO], f32, tag="bb_m")
            bb_e = aprep.tile([1, HO], f32, tag="bb_e")
            nc.scalar.dma_start(bb_m[:], BMU[:][None, :])
            nc.scalar.dma_start(bb_s[:], BSG[:][None, :])
            nc.scalar.dma_start(bb_e[:], EPB[:][None, :])
            nc.vector.tensor_mul(bb_s[:], bb_s[:], bb_e[:])
            nc.vector.tensor_add(bb_s[:], bb_s[:], bb_m[:])

            for ic in range(8):
                m_t = slabl.tile([128, HO], f32, tag="smu")
                s_t = slabl.tile([128, HO], f32, tag="ssg")
                e_t = slabl.tile([128, HO], f32, tag="sep")
                nc.scalar.dma_start(m_t[:], MUS[ic * 128:(ic + 1) * 128, :])
                nc.scalar.dma_start(s_t[:], SGS[ic * 128:(ic + 1) * 128, :])
                nc.scalar.dma_start(e_t[:], EPS[ic * 128:(ic + 1) * 128, :])
                nc.vector.tensor_mul(s_t[:], s_t[:], e_t[:])
                nc.vector.tensor_add(slab_sb[:, ic, :], m_t[:], s_t[:])

        # ---------- phase 0+1: X^T build, then QKV shard matmuls ----------
        with tc.tile_pool(name="pa_big", bufs=1) as pab, \
             tc.tile_pool(name="wload", bufs=6) as wload, \
             tc.tile_pool(name="wcast", bufs=6) as wcast, \
             tc.tile_pool(name="wtp", bufs=4) as wtp, \
             tc.tile_pool(name="qk", bufs=4) as qk:

            bq_sb = sm.tile([128, 8], f32, tag="bq_sb")
            nc.scalar.dma_start(bq_sb[:], BQ[:].rearrange("(c p) -> p c", p=128))
            bk_sb = sm.tile([128, 8], f32, tag="bk_sb")
            nc.scalar.dma_start(bk_sb[:], BK[:].rearrange("(c p) -> p c", p=128))
            bv_sb = sm.tile([128, 8], f32, tag="bv_sb")
            nc.scalar.dma_start(bv_sb[:], BV[:].rearrange("(c p) -> p c", p=128))

            # xt[d%128, d//128, h*128+m] = X[h*128+m, d] in bf16
            xt = pab.tile([128, NCH, 256], bf, tag="xt")

            with tc.tile_pool(name="ptr", bufs=2, space="PSUM") as ptrp, \
                 tc.tile_pool(name="pacc", bufs=2, space="PSUM") as paccp, \
                 tc.tile_pool(name="ps", bufs=1, space="PSUM") as psp, \
                 tc.tile_pool(name="pp", bufs=1, space="PSUM") as ppp, \
                 tc.tile_pool(name="xbp", bufs=1) as xbp:

                # --- X: per-macro load -> cast -> transpose -> copy ---
                # h==0 casts on DVE: the ACT sequencer is still draining the
                # prefetch DMA issues at t=0, so DVE feeds PE immediately.
                xb = xbp.tile([128, 2, DP], bf, tag="xb")
                for h in range(2):
                    for im, (c0, nch) in enumerate(MACROS):
                        xl = wload.tile([128, 1024], f32, tag="wl")
                        nc.sync.dma_start(xl[:, :nch * 128],
                                          X[h * 128:(h + 1) * 128, c0 * 128:(c0 + nch) * 128])
                        if h == 1:
                            nc.scalar.copy(out=xb[:, h, c0 * 128:(c0 + nch) * 128],
                                           in_=xl[:, :nch * 128])
                        else:
                            nc.vector.tensor_copy(out=xb[:, h, c0 * 128:(c0 + nch) * 128],
                                                  in_=xl[:, :nch * 128])
                        ptr = ptrp.tile([128, 8, 128], bf, tag="ptr")
                        for c in range(nch):
                            nc.tensor.transpose(ptr[:, c, :], xb[:, h, (c0 + c) * 128:(c0 + c + 1) * 128], idb[:])
                        nc.vector.tensor_copy(out=xt[:, c0:c0 + nch, h * 128:(h + 1) * 128],
                                              in_=ptr[:, 0:nch, :])

                # --- QKV streaming: flat software pipeline over
                # (ic, mat, macro) units; unit u's matmuls are emitted after
                # unit u+1's transposes so the in-order PE never waits on the
                # DVE copy-back of wt, including across mat/ic boundaries.
                s_ps = psp.tile([128, 2, 256], f32, tag="s_ps")
                # P partial: 512-wide halves so each [*, h, 0:260] slice is
                # bank-aligned (1040B used of each 2KB bank)
                p_ps = ppp.tile([128, 2, 512], f32, tag="p_ps")

                MATS = (("q", Wq, bq_sb), ("k", Wk, bk_sb), ("v", Wv, bv_sb))
                qt_tiles = {}
                pending = None  # (acc, c0, nch, wt, ic, mat, bias_sb, is_last)

                def flush(p):
                    acc, c0, nch, wt, ic, mat, bias_sb, is_last = p
                    for c in range(nch):
                        nc.tensor.matmul(acc[:], lhsT=wt[:, c, :], rhs=xt[:, c0 + c, :],
                                         start=(c0 + c == 0), stop=(c0 + c == NCH - 1))
                    if not is_last:
                        return
                    if mat == "v":
                        nc.scalar.activation(out=vt_sb[:, ic, :], in_=acc[:],
                                             func=AF.Identity,
                                             bias=bias_sb[:, ic:ic + 1], scale=1.0)
                        qtq = qt_tiles.pop((ic, "q"))
                        qtk = qt_tiles.pop((ic, "k"))
                        for h in range(2):
                            nc.tensor.matmul(s_ps[:, h, :], lhsT=qtq[:, h * 128:(h + 1) * 128],
                                             rhs=qtk[:], start=(ic == 0 and h == 0),
                                             stop=(ic == 7 and h == 1))
                            nc.tensor.matmul(p_ps[:, h, 0:HO], lhsT=vt_sb[:, ic, h * 128:(h + 1) * 128],
                                             rhs=slab_sb[:, ic, :], start=(ic == 0),
                                             stop=(ic == 7), skip_group_check=True)
                    else:
                        qt = qk.tile([128, 256], bf, tag="qt")
                        nc.scalar.activation(out=qt[:], in_=acc[:], func=AF.Identity,
                                             bias=bias_sb[:, ic:ic + 1], scale=1.0)
                        qt_tiles[(ic, mat)] = qt

                for ic in range(8):
                    for mat, wsrc, bias_sb in MATS:
                        acc = paccp.tile([128, 256], f32, tag="pacc")
                        for im, (c0, nch) in enumerate(MACROS):
                            wl = wload.tile([128, 1024], f32, tag="wl")
                            nc.sync.dma_start(wl[:, :nch * 128],
                                              wsrc[ic * 128:(ic + 1) * 128, c0 * 128:(c0 + nch) * 128])
                            wc = wcast.tile([128, 1024], bf, tag="wc")
                            if (ic == 0 and mat == "q") or im % 3 == 0:
                                nc.vector.tensor_copy(out=wc[:, :nch * 128], in_=wl[:, :nch * 128])
                            else:
                                nc.scalar.copy(out=wc[:, :nch * 128], in_=wl[:, :nch * 128])
                            ptr = ptrp.tile([128, 8, 128], bf, tag="ptr")
                            for c in range(nch):
                                nc.tensor.transpose(ptr[:, c, :], wc[:, c * 128:(c + 1) * 128], idb[:])
                            wt = wtp.tile([128, 8, 128], bf, tag="wt")
                            nc.vector.tensor_copy(out=wt[:, 0:nch, :], in_=ptr[:, 0:nch, :])
                            if pending is not None:
                                flush(pending)
                            pending = (acc, c0, nch, wt, ic, mat, bias_sb,
                                       im == len(MACROS) - 1)
                flush(pending)

                # ---------- phase 2a: fused AllReduce of [S | P] ----------
                sp_in = dram.tile([M, 516], f32, tag="sp_in")
                sp_out = dram.tile([M, 516], f32, tag="sp_out", addr_space="Shared")
                sp_sb = sm.tile([128, 2, 516], f32, tag="sp_sb")
                nc.scalar.copy(out=sp_sb[:, :, 0:256], in_=s_ps[:])
                nc.scalar.copy(out=sp_sb[:, :, 256:516], in_=p_ps[:, :, 0:HO])
                nc.sync.dma_start(sp_in[:].rearrange("(h p) f -> p h f", p=128), sp_sb[:])
                nc.gpsimd.collective_compute("AllReduce", ALU.add, replica_groups=RG,
                                             ins=[sp_in[:].opt()], outs=[sp_out[:].opt()])

        # big phase-1 pools closed: load scan A table now
        abig = ctx.enter_context(tc.tile_pool(name="abig", bufs=1))
        a_p0 = abig.tile([1, HO * HO], bf, tag="a_p0")
        nc.sync.dma_start(a_p0[:], a_dram[:].rearrange("a b -> (a b)")[None, :])

        spr = sm.tile([128, 2, 516], f32, tag="spr")
        nc.sync.dma_start(spr[:], sp_out[:].rearrange("(h p) f -> p h f", p=128))

        # ---------- phase 2b: softmax rows + a_bar + base ----------
        ex = sm.tile([128, 2, 256], f32, tag="ex")
        mx = sm.tile([128, 2], f32, tag="mx")
        nm = sm.tile([128, 2], f32, tag="nm")
        rs = sm.tile([128, 2], f32, tag="rs")
        inv = sm.tile([128, 2], f32, tag="inv")
        for h in range(2):
            nc.vector.tensor_reduce(mx[:, h:h + 1], spr[:, h, 0:256], axis=AX.X, op=ALU.max)
            nc.vector.tensor_scalar_mul(nm[:, h:h + 1], mx[:, h:h + 1], -SCALE)
            nc.scalar.activation(out=ex[:, h, :], in_=spr[:, h, 0:256], func=AF.Exp,
                                 bias=nm[:, h:h + 1], scale=SCALE,
                                 accum_out=rs[:, h:h + 1])
            nc.vector.reciprocal(inv[:, h:h + 1], rs[:, h:h + 1])
            nc.vector.tensor_scalar_mul(ex[:, h, :], ex[:, h, :], inv[:, h:h + 1])

        with tc.tile_pool(name="psm", bufs=2, space="PSUM") as psmp:
            # a_bar as columns: abt[p, mc] = sum_m attn[m, mc*128+p] / M
            abt_ps = psmp.tile([128, 2], f32, tag="abt")
            for mc in range(2):
                for h in range(2):
                    nc.tensor.matmul(abt_ps[:, mc:mc + 1], lhsT=ex[:, h, mc * 128:(mc + 1) * 128],
                                     rhs=ones_f[:], start=(h == 0), stop=(h == 1))
            abt_sb = sm.tile([128, 2], f32, tag="abt_sb")
            nc.scalar.mul(out=abt_sb[:], in_=abt_ps[:], mul=1.0 / M)
            # base = a_bar @ P
            base_ps = psmp.tile([1, HO], f32, tag="base_ps")
            for mc in range(2):
                nc.tensor.matmul(base_ps[0:1, :], lhsT=abt_sb[:, mc:mc + 1],
                                 rhs=spr[:, mc, 256:516], start=(mc == 0), stop=(mc == 1))
            nc.vector.tensor_add(pre_sb[:], base_ps[0:1, :], bb_s[:])

        # ---------- phase 3: 260-step scan ----------
        # v[j] = tanh(v[j-1]*A[j-1,j] + pre[j]). Each node's remaining
        # contributions split: DVE updates the near columns [j+2, j+KN)
        # directly in pre_sb (on the 2-node critical cycle), while PE
        # accumulates the far columns [j+KN, 260) as rank-1 updates into a
        # PSUM row; finished column blocks are folded into pre_sb by one DVE
        # add well before the tanh chain reaches them.
        KN, BC = 24, 16
        far_js = [j for j in range(HO) if j + KN < HO]
        consol_at = {}
        b = KN
        while b < HO:
            be = min(b + BC, HO)
            consol_at.setdefault(be - 1 - KN, []).append((b, be))
            b = be
        with tc.tile_pool(name="psc", bufs=1, space="PSUM") as pscp:
            ps_far = pscp.tile([1, HO], f32, tag="ps_far")
            for j in range(HO):
                if j == 0:
                    nc.scalar.activation(out=vrow[0:1, 0:1], in_=pre_sb[0:1, 0:1],
                                         func=AF.Tanh)
                else:
                    nc.scalar.activation(out=vrow[0:1, j:j + 1], in_=vrow[0:1, j - 1:j],
                                         func=AF.Tanh,
                                         scale=band[0:1, j - 1:j, 0],
                                         bias=pre_sb[0:1, j:j + 1])
                if j + KN < HO:
                    nc.tensor.matmul(ps_far[0:1, j + KN:HO], lhsT=vrow[0:1, j:j + 1],
                                     rhs=a_p0[0:1, j * HO + j + KN:(j + 1) * HO],
                                     start=(j == 0), stop=(j == far_js[-1]),
                                     skip_group_check=True)
                jl, jr = j + 2, min(j + KN, HO)
                if jl < jr:
                    nc.vector.scalar_tensor_tensor(
                        out=pre_sb[0:1, jl:jr],
                        in0=a_p0[0:1, j * HO + jl:j * HO + jr],
                        scalar=vrow[0:1, j:j + 1],
                        in1=pre_sb[0:1, jl:jr],
                        op0=ALU.mult, op1=ALU.add)
                for (cb, ce) in consol_at.get(j, []):
                    nc.vector.tensor_add(pre_sb[0:1, cb:ce], ps_far[0:1, cb:ce],
                                         pre_sb[0:1, cb:ce])
            nc.scalar.copy(out=y4[:], in_=vrow[0:1, HO - 4:HO])
        nc.sync.dma_start(Y[:], y4[:])

    nc.compile()
    return nc


def _get_nc():
    if "nc" not in _CACHE:
        _CACHE["nc"] = _build()
    return _CACHE["nc"]


def _make_in_maps(inputs):
    X = np.asarray(inputs["input_matrix"], np.float32)
    Wq = np.asarray(inputs["Wq"], np.float32)
    Wk = np.asarray(inputs["Wk"], np.float32)
    Wv = np.asarray(inputs["Wv"], np.float32)
    bq = np.asarray(inputs["bq"], np.float32)
    bk = np.asarray(inputs["bk"], np.float32)
    bv = np.asarray(inputs["bv"], np.float32)
    mu = np.asarray(inputs["weight_mu"], np.float32)
    sg = np.asarray(inputs["weight_sigma"], np.float32)
    ep = np.asarray(inputs["eps_w"], np.float32)
    bmu = np.asarray(inputs["bias_mu"], np.float32)
    bsg = np.asarray(inputs["bias_sigma"], np.float32)
    epb = np.asarray(inputs["eps_b"], np.float32)

    Xp = np.zeros((M, DP), np.float32)
    Xp[:, :D] = X

    muA = np.ascontiguousarray(mu[D:NTOT, D:NTOT])
    sgA = np.ascontiguousarray(sg[D:NTOT, D:NTOT])
    epA = np.ascontiguousarray(ep[D:NTOT, D:NTOT])

    in_maps = []
    for c in range(NCORES):
        st, sz = STARTS[c], SIZES[c]

        def rows2d_pad(A):
            out = np.zeros((SH, DP), np.float32)
            out[:sz, :D] = A[st:st + sz]
            return out

        def rows2d(A):
            out = np.zeros((SH, A.shape[1]), np.float32)
            out[:sz] = A[st:st + sz]
            return out

        def rows1d(a):
            out = np.zeros((SH,), np.float32)
            out[:sz] = a[st:st + sz]
            return out

        in_maps.append({
            "x": Xp,
            "wq": rows2d_pad(Wq), "wk": rows2d_pad(Wk), "wv": rows2d_pad(Wv),
            "bq": rows1d(bq), "bk": rows1d(bk), "bv": rows1d(bv),
            "mus": rows2d(mu[:, D:NTOT]),
            "sgs": rows2d(sg[:, D:NTOT]),
            "eps": rows2d(ep[:, D:NTOT]),
            "mua": muA, "sga": sgA, "epa": epA,
            "bmu": bmu, "bsg": bsg, "epb": epb,
        })
    return in_maps


def kernel(**inputs):
    from concourse.bass_utils import run_bass_kernel_spmd

    nc = _get_nc()
    in_maps = _make_in_maps(inputs)
    res = run_bass_kernel_spmd(nc, in_maps, core_ids=list(range(NCORES)))
    return np.asarray(res.results[0]["y"], np.float32).reshape(4)
